# revision 2
# baseline (speedup 1.0000x reference)
"""GNN message passing (nn_OPID_78769700208710) on 8 TRN2 NeuronCores — v5.

HW probing showed a hard ~174 GB/s per-core HBM ceiling in this environment
(same with 1 or 8 cores active), so the A-stream bytes are the whole game:
  * RESP=12 resident pairs (24 of 160 windows) — SBUF partition is 224KB.
  * src rows >= 20000 are structurally zero -> windows 157-159 never touch
    the PE or the wire (window 156 streamed as a single panel).
  * (from v4) DMA issues alternate the two HWDGE rings; one exchange scatter
    DMA; decode DMAs batched.
"""

import numpy as np

N = 20000
NP = 20480
W = 160            # src windows of 128
B = 64
CORES = 8
NLOC = NP // CORES  # 2560 dst per core
WLOC = NLOC // 128  # 20 dst windows per core
NCHUNK = NLOC // 512  # 5
NPAIR = 80          # window pairs
RESP = 11           # resident pairs (windows 0..21)
H = 64
STEPS = 6
SIGNS = (1.0, -1.0, 1.0, -1.0, 1.0, -1.0)

_CACHE = {}


def _np_softplus(x):
    return np.log1p(np.exp(-np.abs(x))) + np.maximum(x, 0.0)


def _np_sigmoid(x):
    return 1.0 / (1.0 + np.exp(-x))


def _build_program(reps=1, debug=False, compile_=True):
    key = ("nc", reps, debug)
    if key in _CACHE:
        return _CACHE[key]

    import concourse.bacc as bacc
    import concourse.mybir as mybir
    from concourse import tile

    f16 = mybir.dt.float16
    f32 = mybir.dt.float32
    AF = mybir.ActivationFunctionType
    OP = mybir.AluOpType

    nc = bacc.Bacc(
        "TRN2",
        target_bir_lowering=False,
        debug=False,
        enable_asserts=False,
        num_devices=CORES,
    )

    a16p = nc.dram_tensor("a16p", [NPAIR, 128, 2 * NLOC], f16, kind="ExternalInput")
    h16i = nc.dram_tensor("h16i", [128, W * B], f16, kind="ExternalInput")
    h0s = nc.dram_tensor("h0s", [B, NLOC], f16, kind="ExternalInput")
    xstat = nc.dram_tensor("xstat", [8, B * NLOC // 2], f16, kind="ExternalInput")
    w18x = nc.dram_tensor("w18x", [128, 128], f16, kind="ExternalInput")
    w2p = nc.dram_tensor("w2p", [128, 2], f16, kind="ExternalInput")
    ident = nc.dram_tensor("ident", [64, 64], f16, kind="ExternalInput")
    alph = nc.dram_tensor("alph", [128, 2 * STEPS], f32, kind="ExternalInput")
    y = nc.dram_tensor("y", [B, NLOC], f32, kind="ExternalOutput")

    NTOKH = B * NLOC // 2  # 81920 tokens per half (b<32 / b>=32)
    NITER = NTOKH // (4 * 512)  # 40 decode iterations
    XB = 4                  # decode iterations per x8t load

    with tile.TileContext(nc) as tc:
        with (
            tc.tile_pool(name="const", bufs=1) as constp,
            tc.tile_pool(name="apan", bufs=3) as apanp,
            tc.tile_pool(name="ep", bufs=2) as epp,
            tc.tile_pool(name="dec", bufs=2) as decp,
            tc.tile_pool(name="dram", bufs=1, space="DRAM") as dramp,
        ):
            dmae = [nc.sync, nc.scalar]   # the two HWDGE rings

            # ---- persistent SBUF state ----
            ares = [constp.tile([128, 2 * NLOC], f16, tag=f"ares{p}", name=f"ares{p}")
                    for p in range(RESP)]
            h16_sb = constp.tile([128, W * B], f16, tag="h16sb")
            h0s_sb = constp.tile([B, NLOC], f16, tag="h0s")
            alph_sb = constp.tile([128, 2 * STEPS], f32, tag="alph")
            w18_sb = constp.tile([128, 128], f16, tag="w18")
            w2p_sb = constp.tile([128, 2], f16, tag="w2p")
            ident_sb = constp.tile([64, 64], f16, tag="ident")
            h6_sb = constp.tile([B, NLOC], f16, tag="h6")
            h16s = constp.tile([128, WLOC * B], f16, tag="h16s")

            for p in range(RESP):
                dmae[p % 2].dma_start(ares[p][:], a16p.ap()[p])
            nc.sync.dma_start(h0s_sb[:], h0s.ap())
            nc.sync.dma_start(alph_sb[:], alph.ap())
            nc.sync.dma_start(w18_sb[:], w18x.ap())
            nc.sync.dma_start(w2p_sb[:], w2p.ap())
            nc.sync.dma_start(ident_sb[:], ident.ap())

            # DRAM bounce buffers
            bi = dramp.tile([128, WLOC * B], f16, tag="bi")
            bos = [
                dramp.tile([CORES, 128, WLOC * B], f16, tag=f"bo{r}_{k}",
                           name=f"bo{r}_{k}", addr_space="Shared")
                for r in range(reps) for k in range(STEPS - 1)
            ]
            x8d = dramp.tile([8, NTOKH], f16, tag="x8d")

            nc.sync.dma_start(x8d[:], xstat.ap())

            # stream order: full pairs 12..77, then window 156 alone (157-159
            # are zero rows), then residents
            pair_order = list(range(RESP, 78)) + [78] + list(range(RESP))
            NW_REAL = (78 - RESP) * 2 + 1 + 2 * RESP  # windows with nonzero A

            y2 = y.ap().rearrange("(p b) n -> b p n", p=2)

            for rep in range(reps):
                nc.sync.dma_start(h16_sb[:], h16i.ap())

                prop = tc.tile_pool(name=f"psprop{rep}", bufs=1, space="PSUM")
                psp = prop.__enter__()
                for k in range(STEPS):
                    ps = [psp.tile([B, 512], f32, tag=f"ps{i}", name=f"ps{i}")
                          for i in range(NCHUNK)]

                    nw_done = 0
                    for p in pair_order:
                        nj = 1 if p == 78 else 2
                        if p < RESP:
                            src = ares[p]
                        elif p == 78:
                            src = apanp.tile([128, 2 * NLOC], f16, tag="apan")
                            dmae[p % 2].dma_start(
                                src[:, :NLOC], a16p.ap()[p][:, :NLOC])
                        else:
                            src = apanp.tile([128, 2 * NLOC], f16, tag="apan")
                            dmae[p % 2].dma_start(src[:], a16p.ap()[p])
                        for j in range(nj):
                            w_g = 2 * p + j
                            lhsT = h16_sb[:, w_g * B : (w_g + 1) * B]
                            for c5 in range(NCHUNK):
                                nc.tensor.matmul(
                                    ps[c5][:, :],
                                    lhsT=lhsT,
                                    rhs=src[:, j * NLOC + c5 * 512 : j * NLOC + (c5 + 1) * 512],
                                    start=nw_done == 0,
                                    stop=nw_done == NW_REAL - 1,
                                )
                            nw_done += 1

                    # ---- epilogue ----
                    if k < STEPS - 1:
                        for c5 in range(NCHUNK):
                            h0a = epp.tile([B, 512], f32, tag="h0a")
                            nc.scalar.activation(
                                h0a[:],
                                h0s_sb[:, c5 * 512 : (c5 + 1) * 512],
                                AF.Copy,
                                scale=alph_sb[:B, k : k + 1],
                            )
                            ht16 = epp.tile([B, 512], f16, tag="ht16")
                            nc.vector.scalar_tensor_tensor(
                                ht16[:],
                                ps[c5][:, :],
                                alph_sb[:B, STEPS + k : STEPS + k + 1],
                                h0a[:],
                                OP.mult,
                                OP.add,
                            )
                            for d in range(4):
                                w_loc = 4 * c5 + d
                                psT = psp.tile([128, B], f16, tag="psT", bufs=2)
                                nc.tensor.transpose(
                                    psT[:],
                                    ht16[:, d * 128 : (d + 1) * 128],
                                    ident_sb[:],
                                )
                                nc.vector.tensor_copy(
                                    h16s[:, w_loc * B : (w_loc + 1) * B], psT[:]
                                )
                        # exchange: slice -> DRAM -> AllGather -> one scatter
                        nc.sync.dma_start(bi[:], h16s[:])
                        bo = bos[rep * (STEPS - 1) + k]
                        nc.gpsimd.collective_compute(
                            "AllGather",
                            OP.bypass,
                            replica_groups=[list(range(CORES))],
                            ins=[bi.opt()],
                            outs=[bo.opt()],
                        )
                        nc.sync.dma_start(
                            h16_sb[:].rearrange("p (c f) -> p c f", c=CORES),
                            bo[:].rearrange("c p f -> p c f"),
                        )
                    else:
                        # final step: h6 fp16 in batch layout for decode
                        for c5 in range(NCHUNK):
                            h0a = epp.tile([B, 512], f32, tag="h0a")
                            nc.scalar.activation(
                                h0a[:],
                                h0s_sb[:, c5 * 512 : (c5 + 1) * 512],
                                AF.Copy,
                                scale=alph_sb[:B, k : k + 1],
                            )
                            nc.vector.scalar_tensor_tensor(
                                h6_sb[:, c5 * 512 : (c5 + 1) * 512],
                                ps[c5][:, :],
                                alph_sb[:B, STEPS + k : STEPS + k + 1],
                                h0a[:],
                                OP.mult,
                                OP.add,
                            )
                        nc.sync.dma_start(
                            x8d[2].rearrange("(b n) -> b n", b=B // 2),
                            h6_sb[: B // 2, :],
                        )
                        nc.scalar.dma_start(
                            x8d[6].rearrange("(b n) -> b n", b=B // 2),
                            h6_sb[B // 2 :, :],
                        )
                prop.__exit__(None, None, None)

                # ---------------- decode ----------------
                # group g covers chunks c = 40g + it (it = 0..39):
                #   x8t loads batch XB iterations; y rows batch 5 iterations
                decps = tc.tile_pool(name=f"psdec{rep}", bufs=1, space="PSUM")
                psd = decps.__enter__()
                x8t = None
                ysb5 = None
                for it in range(NITER):
                    ib, io = divmod(it, XB)
                    if io == 0:
                        x8t = decp.tile([128, XB * 512], f16, tag="x8t")
                        for g in range(4):
                            c0 = 40 * g + XB * ib
                            dmae[g % 2].dma_start(
                                x8t[32 * g : 32 * g + 8, :],
                                x8d[:, 512 * c0 : 512 * (c0 + XB)],
                            )
                    yb, yo = divmod(it, NCHUNK)
                    if yo == 0:
                        ysb5 = decp.tile([128, NLOC], f32, tag="ysb", bufs=1)
                    psd2 = psd.tile([128, 512], f32, tag="psd2", bufs=2)
                    for g in range(4):
                        psd1 = psd.tile([128, 512], f32, tag=f"psd1_{g}",
                                        name=f"psd1_{g}")
                        nc.tensor.matmul(
                            psd1[:],
                            lhsT=w18_sb[32 * g : 32 * g + 8, :],
                            rhs=x8t[32 * g : 32 * g + 8, io * 512 : (io + 1) * 512],
                            start=True,
                            stop=True,
                            tile_position=(32 * g, 0),
                        )
                        hds = decp.tile([128, 512], f16, tag=f"hds{g}",
                                        name=f"hds{g}", bufs=1)
                        if g % 2 == 0:
                            nc.scalar.activation(hds[:], psd1[:], AF.Relu)
                        else:
                            nc.vector.tensor_scalar_max(hds[:], psd1[:], 0.0)
                        nc.tensor.matmul(
                            psd2[32 * g : 32 * g + 2, :],
                            lhsT=w2p_sb[:],
                            rhs=hds[:],
                            start=True,
                            stop=True,
                            tile_position=(0, 32 * g),
                        )
                        nc.vector.tensor_copy(
                            ysb5[32 * g : 32 * g + 2, yo * 512 : (yo + 1) * 512],
                            psd2[32 * g : 32 * g + 2, :],
                        )
                    if yo == NCHUNK - 1:
                        for g in range(4):
                            dmae[g % 2].dma_start(
                                y2[8 * g + yb],
                                ysb5[32 * g : 32 * g + 2, :],
                            )
                decps.__exit__(None, None, None)

    if compile_:
        nc.compile()
    _CACHE[key] = nc
    return nc


def _prep_inputs(
    ctl_base, u_raw, g_logits, alpha_logits, cell_emb,
    W1, b1, W2, b2, edge_val, edge_src, edge_dst, cell_idx,
):
    g = _np_softplus(np.asarray(g_logits, np.float64))
    alphas = _np_sigmoid(np.asarray(alpha_logits, np.float64))

    A = np.zeros((NP, NP), np.float32)
    for r in range(6):
        w = (SIGNS[r] * g[r]) * np.asarray(edge_val[r], np.float64)
        np.add.at(A, (np.asarray(edge_src[r]), np.asarray(edge_dst[r])),
                  w.astype(np.float32))

    u_pad = np.zeros((B, NP), np.float32)
    u_pad[:, :N] = u_raw
    ctl_pad = np.zeros((B, NP), np.float32)
    ctl_pad[:, :N] = ctl_base

    # initial h in full window layout fp16
    h16i_np = np.ascontiguousarray(
        u_pad.reshape(B, W, 128).transpose(2, 1, 0).reshape(128, W * B)
    ).astype(np.float16)

    alph_np = np.zeros((128, 2 * STEPS), np.float32)
    alph_np[:, :STEPS] = alphas.astype(np.float32)
    alph_np[:, STEPS:] = (1.0 - alphas).astype(np.float32)

    W1b = np.zeros((4, H), np.float32)
    W1b[0] = W1[0]
    W1b[1] = W1[1]
    W1b[2] = W1[2]
    W1b[3] = b1
    base8 = np.zeros((8, 128), np.float32)
    base8[0:4, 0:64] = W1b
    base8[4:8, 64:128] = W1b
    w18x_np = np.zeros((128, 128), np.float16)
    for gg in range(4):
        w18x_np[32 * gg : 32 * gg + 8, :] = base8.astype(np.float16)

    w2p_np = np.zeros((128, 2), np.float16)
    w2p_np[0:64, 0] = W2[:, 0].astype(np.float16)
    w2p_np[64:128, 1] = W2[:, 0].astype(np.float16)

    ident_np = np.eye(64, dtype=np.float16)

    A16 = A.astype(np.float16)
    in_maps = []
    for c in range(CORES):
        sl = slice(c * NLOC, (c + 1) * NLOC)
        Acs = A16[:, sl]  # [NP, NLOC]
        a16p_c = np.ascontiguousarray(
            Acs.reshape(NPAIR, 2, 128, NLOC)
            .transpose(0, 2, 1, 3)
            .reshape(NPAIR, 128, 2 * NLOC)
        )
        h0s_c = np.ascontiguousarray(u_pad[:, sl]).astype(np.float16)
        xstat_c = np.zeros((8, B * NLOC // 2), np.float16)
        xstat_c[0] = ctl_pad[: B // 2, sl].reshape(-1).astype(np.float16)
        xstat_c[1] = u_pad[: B // 2, sl].reshape(-1).astype(np.float16)
        xstat_c[3] = np.float16(1.0)
        xstat_c[4] = ctl_pad[B // 2 :, sl].reshape(-1).astype(np.float16)
        xstat_c[5] = u_pad[B // 2 :, sl].reshape(-1).astype(np.float16)
        xstat_c[7] = np.float16(1.0)
        in_maps.append(
            {
                "a16p": a16p_c,
                "h16i": h16i_np,
                "h0s": h0s_c,
                "xstat": xstat_c,
                "w18x": w18x_np,
                "w2p": w2p_np,
                "ident": ident_np,
                "alph": alph_np,
            }
        )

    cemb_rows = np.asarray(cell_emb)[np.asarray(cell_idx)]
    ybias = (
        cemb_rows.astype(np.float64) @ np.asarray(W2, np.float64)[:, 0]
        + float(np.asarray(b2).reshape(-1)[0])
    ).astype(np.float32)
    return in_maps, ybias


def kernel(
    ctl_base, u_raw, g_logits, alpha_logits, cell_emb,
    W1, b1, W2, b2, edge_val, edge_src, edge_dst, cell_idx,
):
    from concourse.bass_utils import run_bass_kernel_spmd

    in_maps, ybias = _prep_inputs(
        np.asarray(ctl_base), np.asarray(u_raw), np.asarray(g_logits),
        np.asarray(alpha_logits), np.asarray(cell_emb), np.asarray(W1),
        np.asarray(b1), np.asarray(W2), np.asarray(b2), np.asarray(edge_val),
        np.asarray(edge_src), np.asarray(edge_dst), np.asarray(cell_idx),
    )
    nc = _build_program()
    _CACHE["in_maps"] = in_maps
    _CACHE["ybias"] = ybias
    res = run_bass_kernel_spmd(nc, in_maps, core_ids=list(range(CORES)))
    out = np.concatenate([res.results[c]["y"] for c in range(CORES)], axis=1)
    out = out + ybias[:, None]
    return np.ascontiguousarray(out[:, :N]).astype(np.float32)
